# revision 3
# baseline (speedup 1.0000x reference)
"""Circular-convolution helper kernel for Trainium2 (8 NeuronCores).

Math: out[i] = sum_b sum_t x1[b,(i-t)%D] * x2[b,t]
            = sum_j G[j, (i-j)%D]   where G = x1^T @ x2  ([D, D], K=B contraction)

Sharding: G's rows are split across the 8 cores (core c owns rows
[128c, 128c+128)).  Each core computes its row-block A = x1c^T @ x2 with a
single K=128 matmul, then reduces A's circular anti-diagonals:
    part_c[i] = sum_m A[m, (i-m)%D]
via a doubled DRAM scratch (write A twice side by side, read back with a
row-stride of 2D-1 which realigns the diagonals into columns), followed by a
ones-vector matmul to collapse the 128 partitions.  The host rotates each
core's partial by 128c and sums.
"""

import numpy as np

B = 128
DIM = 1024
NCORES = 8
CHUNK = DIM // NCORES  # 128 rows of G per core
NHALF = 512  # PSUM bank = 512 fp32


_cached = {}


def _build():
    if "nc" in _cached:
        return _cached["nc"]

    import concourse.bass as bass
    import concourse.mybir as mybir
    from concourse import bacc
    from concourse.tile import TileContext

    f32 = mybir.dt.float32

    nc = bacc.Bacc("TRN2", target_bir_lowering=False, debug=False)

    x1c = nc.dram_tensor("x1c", [B, CHUNK], f32, kind="ExternalInput")
    x2 = nc.dram_tensor("x2", [B, DIM], f32, kind="ExternalInput")
    out = nc.dram_tensor("out", [1, DIM], f32, kind="ExternalOutput")
    # doubled scratch: gd[m, k] = A[m, k % D]
    gd = nc.dram_tensor("gd", [CHUNK, 2 * DIM], f32, kind="Internal")

    with TileContext(nc) as tc:
        with (
            tc.tile_pool(name="sb", bufs=1) as sb,
            tc.tile_pool(name="ps", bufs=1, space="PSUM") as ps,
        ):
            x1t = sb.tile([B, CHUNK], f32)
            nc.sync.dma_start(x1t[:], x1c.ap())
            x2t = sb.tile([B, DIM], f32)
            nc.sync.dma_start(x2t[:], x2.ap())

            # A = x1c^T @ x2  -> [CHUNK, DIM] in PSUM (2 banks)
            g = ps.tile([CHUNK, DIM], f32)
            for h in range(2):
                nc.tensor.matmul(
                    g[:, h * NHALF : (h + 1) * NHALF],
                    x1t[:],
                    x2t[:, h * NHALF : (h + 1) * NHALF],
                    start=True,
                    stop=True,
                )

            # PSUM -> SBUF (one bank per engine, in parallel)
            a = sb.tile([CHUNK, DIM], f32)
            nc.scalar.copy(a[:, 0:NHALF], g[:, 0:NHALF])
            nc.vector.tensor_copy(a[:, NHALF:DIM], g[:, NHALF:DIM])

            # write A twice side by side -> gd[m, k] = A[m, k % D]
            gd_ap = gd.ap()
            nc.sync.dma_start(gd_ap[:, 0:DIM], a[:])
            nc.sync.dma_start(gd_ap[:, DIM : 2 * DIM], a[:])

            # diagonal read: H[m, i] = gd[m, i + D - m] = A[m, (i - m) % D]
            # flat addr = m*(2D) + i + D - m = D + m*(2D-1) + i
            ht = sb.tile([CHUNK, DIM], f32)
            diag = bass.AP(gd, DIM, [[2 * DIM - 1, CHUNK], [1, DIM]])
            nc.sync.dma_start(ht[:], diag)

            # collapse partitions: part[i] = sum_m H[m, i]
            ones = sb.tile([CHUNK, 1], f32)
            nc.vector.memset(ones[:], 1.0)
            o = ps.tile([1, DIM], f32)
            for h in range(2):
                nc.tensor.matmul(
                    o[:, h * NHALF : (h + 1) * NHALF],
                    ones[:],
                    ht[:, h * NHALF : (h + 1) * NHALF],
                    start=True,
                    stop=True,
                )

            ot = sb.tile([1, DIM], f32)
            nc.scalar.copy(ot[:], o[:])
            nc.sync.dma_start(out.ap(), ot[:])

    nc.compile()
    _cached["nc"] = nc
    return nc


def _in_maps(input1, input2):
    x1 = np.ascontiguousarray(np.asarray(input1, dtype=np.float32))
    x2 = np.ascontiguousarray(np.asarray(input2, dtype=np.float32))
    return [
        {
            "x1c": np.ascontiguousarray(x1[:, c * CHUNK : (c + 1) * CHUNK]),
            "x2": x2,
        }
        for c in range(NCORES)
    ]


def _combine(results):
    total = np.zeros(DIM, np.float64)
    for c in range(NCORES):
        total += np.roll(results[c]["out"][0].astype(np.float64), CHUNK * c)
    return total.astype(np.float32).reshape(1, 1, DIM)


def _run(input1, input2, **kwargs):
    from concourse import bass_utils

    nc = _build()
    res = bass_utils.run_bass_kernel_spmd(
        nc, _in_maps(input1, input2), core_ids=list(range(NCORES)), **kwargs
    )
    return res


def kernel(input1, input2):
    res = _run(input1, input2)
    return _combine(res.results)
